# revision 9
# baseline (speedup 1.0000x reference)
"""Bass/Trainium2 kernel for nn_AdaptiveSparseReservoir (self-contained).

out[b, c] = relu(sum_k x[b, rows[k]] * values[k] for cols[k]==c  + bias[c])
  x [1024, 4096] f32; values [262144] f32; rows/cols [262144] i32;
  bias [4096] f32  ->  out [1024, 4096] f32

Strategy
--------
Densify the sparse COO kernel on the host into W [4096, 4096] (1.6%
density with unstructured support is far too dense for gather/scatter on
TRN2 — a dense bf16 TensorEngine matmul moves ~16x fewer bytes), then run
the dense matmul column-sharded across the 8 NeuronCores with NO
collectives: core i computes outT_i = relu(W[:, 512i:512(i+1)].T @ x.T + b_i).

Measured-on-silicon design points:
- PSUM-accumulating bf16 matmuls (K=128, N=512) retire at ~213 ns — the
  fp32 PSUM read-modify-write path is the per-core floor (~54 us for the
  256 matmuls/core). DMA (12 MB/core) streams at ~310 GB/s when split
  across BOTH HWDGE rings (sync + scalar), ~39 us, and hides under the
  matmul stream. This is the "ridge" regime.
- The output is computed TRANSPOSED so the per-column bias lands on the
  PSUM partition axis: bias+relu is then a single fused op per PSUM bank,
  alternating ScalarE `activation` / VectorE `tensor_scalar`, interleaved
  with the final k-tile's matmuls so the tail drains on three engines.
- Inputs are host-packed partition-major so each input streams as a few
  large perfectly-coalesced DMA chunks (fine-grained first/last chunks
  for early start / early finish); output returns as bf16 (host upcasts).
- TileContext's exit barrier is replaced by a drain-only tail: the Bass
  preamble sem_clears at the start of every execution, so the butterfly
  barrier + semaphore clears (~4 us) are dead weight.
- A few garbage warm-up matmuls run before the first data lands (k=0's
  start=True clear discards them). A forced-cold A/B measured them neutral
  (the HAM ramp penalty doesn't materialize at this matmul cadence); they
  are kept as harmless engine activity during the otherwise idle start.
"""

import os
import types

import numpy as np
import ml_dtypes

D_IN = 4096
UNITS = 4096
NNZ = 262144
BATCH = 1024
N_CORES = 8
N_SHARD = UNITS // N_CORES  # 512 output columns per core
K_TILES = D_IN // 128  # 32
N_TILES = N_SHARD // 128  # 4
M_HALVES = BATCH // 512  # 2
N_WARMUP = 3

_CACHE = {}


def _drain_only(self, tick_clock, wait_clock):
    """Tail = DMA/compute drain only; skip the butterfly barrier + sem
    clears (the Bass preamble sem_clears at the start of each execution,
    and NEFF completion already requires every engine queue to finish)."""
    from concourse.tile import ScopedClock

    drain_inst = self.nc.sync.drain()
    wait_clock.add_sem_waits(
        drain_inst.ins, ScopedClock({None: tick_clock.global_clock})
    )
    popped = self.nc._tile_sem_poison_stack.pop()
    assert popped is self._sem_poison


def _build():
    import concourse.mybir as mybir
    import concourse.tile as tile
    from concourse import bacc

    nc = bacc.Bacc("TRN2", target_bir_lowering=False, debug=False, num_devices=N_CORES)
    bf16 = mybir.dt.bfloat16
    f32 = mybir.dt.float32

    xT_ext = nc.declare_dram_parameter("xT", [128, K_TILES * 1024], bf16, isOutput=False)
    w_ext = nc.declare_dram_parameter("w", [128, K_TILES * 512], bf16, isOutput=False)
    b_ext = nc.declare_dram_parameter("bias", [128, N_TILES], f32, isOutput=False)
    out_ext = nc.declare_dram_parameter("out", [N_SHARD, BATCH], bf16, isOutput=True)

    tc_outer = tile.TileContext(nc)
    try:
        # verify the internals _drain_only touches exist in this concourse
        from concourse.tile import ScopedClock  # noqa: F401

        assert hasattr(tc_outer, "_drain_and_barrier")
        assert hasattr(nc, "_tile_sem_poison_stack")
        tc_outer._drain_and_barrier = types.MethodType(_drain_only, tc_outer)
    except Exception:
        pass  # stock barrier exit: ~4us slower, still correct
    with tc_outer as tc:
        with (
            tc.tile_pool(name="consts", bufs=1) as cpool,
            tc.tile_pool(name="xk", bufs=1) as xpool,
            tc.tile_pool(name="wk", bufs=1) as wpool,
            tc.tile_pool(name="osb", bufs=4) as opool,
            tc.tile_pool(name="psum", bufs=1, space="PSUM") as ppool,
        ):
            psum = [
                ppool.tile([128, 512], f32, tag=f"ps{i}", name=f"ps{i}")
                for i in range(N_TILES * M_HALVES)
            ]

            # PE warm-up against the HAM cold clock; k=0's start=True clear
            # discards the garbage
            warm = cpool.tile([128, 640], bf16)
            nc.vector.memset(warm[:, :], 0)
            for _ in range(N_WARMUP):
                nc.tensor.matmul(
                    psum[0][:, :], warm[:, 0:128], warm[:, 128:640],
                    start=True, stop=True,
                )

            tbl_warm = cpool.tile([128, 1], f32)
            bias_sb = cpool.tile([128, N_TILES], f32)
            nc.sync.dma_start(bias_sb[:, :], b_ext[:, :])

            xts = xpool.tile([128, K_TILES * 1024], bf16, name="xts")
            wts = wpool.tile([128, K_TILES * 512], bf16, name="wts")

            # interleave x/w chunks in k order, alternating HWDGE rings;
            # fine-grained first chunks (early PE start) and last x chunk
            # (early final accumulations)
            xbounds = [0, 1, 2, 4] + list(range(6, K_TILES - 1, 2)) + [K_TILES - 1, K_TILES]
            wbounds = [0, 1, 2, 4] + list(range(8, K_TILES + 1, 4))
            chunks = []  # ("x"|"w", klo, khi) in k-coverage order
            xi = wi = 0
            while xi < len(xbounds) - 1 or wi < len(wbounds) - 1:
                kx = xbounds[xi] if xi < len(xbounds) - 1 else K_TILES
                kw = wbounds[wi] if wi < len(wbounds) - 1 else K_TILES
                if kw <= kx and wi < len(wbounds) - 1:
                    chunks.append(("w", wbounds[wi], wbounds[wi + 1]))
                    wi += 1
                else:
                    chunks.append(("x", xbounds[xi], xbounds[xi + 1]))
                    xi += 1
            for i, (kind, klo, khi) in enumerate(chunks):
                eng = nc.sync if i % 2 == 0 else nc.scalar
                if kind == "x":
                    eng.dma_start(
                        xts[:, klo * 1024 : khi * 1024],
                        xT_ext[:, klo * 1024 : khi * 1024],
                    )
                else:
                    eng.dma_start(
                        wts[:, klo * 512 : khi * 512],
                        w_ext[:, klo * 512 : khi * 512],
                    )

            # trigger the Relu act-table load now (ACT is idle during the
            # stream); bacc hoists LoadActFuncSet before this instruction,
            # keeping the ~1.3us load off the epilogue critical path
            nc.scalar.activation(
                tbl_warm[:, :], warm[:, 0:1], mybir.ActivationFunctionType.Relu
            )

            def mm(k, nt, mh, stop=False):
                nc.tensor.matmul(
                    psum[nt * M_HALVES + mh][:, :],
                    wts[:, k * 512 + nt * 128 : k * 512 + (nt + 1) * 128],
                    xts[:, k * 1024 + mh * 512 : k * 1024 + (mh + 1) * 512],
                    start=(k == 0),
                    stop=stop,
                )

            for k in range(K_TILES - 1):
                for nt in range(N_TILES):
                    for mh in range(M_HALVES):
                        mm(k, nt, mh)

            # last k: interleave each bank's final matmul with its fused
            # bias+relu epilogue, alternating ScalarE/VectorE, storing bf16
            k = K_TILES - 1
            for i, (nt, mh) in enumerate(
                [(nt, mh) for nt in range(N_TILES) for mh in range(M_HALVES)]
            ):
                mm(k, nt, mh, stop=True)
                ot = opool.tile([128, 512], bf16, name=f"ot{i}", tag="ot")
                if i % 2 == 0:
                    nc.scalar.activation(
                        ot[:, :],
                        psum[nt * M_HALVES + mh][:, :],
                        mybir.ActivationFunctionType.Relu,
                        bias=bias_sb[:, nt : nt + 1],
                    )
                else:
                    nc.vector.tensor_scalar(
                        ot[:, :],
                        psum[nt * M_HALVES + mh][:, :],
                        bias_sb[:, nt : nt + 1],
                        0.0,
                        mybir.AluOpType.add,
                        mybir.AluOpType.max,
                    )
                eng = nc.sync if i % 2 == 0 else nc.scalar
                eng.dma_start(
                    out_ext[nt * 128 : (nt + 1) * 128, mh * 512 : (mh + 1) * 512],
                    ot[:, :],
                )

    nc.compile()
    return nc


def _get_nc():
    if "nc" not in _CACHE:
        _CACHE["nc"] = _build()
    return _CACHE["nc"]


def kernel(x, values, bias, rows, cols):
    from concourse.bass_utils import run_bass_kernel_spmd

    x = np.asarray(x, np.float32)
    values = np.asarray(values, np.float32)
    bias = np.asarray(bias, np.float32)
    rows = np.asarray(rows)
    cols = np.asarray(cols)

    # densify via bincount (vectorized scatter-add; duplicates accumulate)
    flat = rows.astype(np.int64) * UNITS + cols.astype(np.int64)
    W = np.bincount(flat, weights=values.astype(np.float64), minlength=D_IN * UNITS)
    W = W.reshape(D_IN, UNITS).astype(np.float32)

    # partition-major xT: xT_pm[p, k*1024 + m] = x[m, k*128 + p]
    xT16 = np.ascontiguousarray(x.T).astype(ml_dtypes.bfloat16)  # [D_IN, BATCH]
    xT_pm = np.ascontiguousarray(
        xT16.reshape(K_TILES, 128, BATCH).transpose(1, 0, 2).reshape(128, K_TILES * BATCH)
    )
    W16 = W.astype(ml_dtypes.bfloat16)

    in_maps = []
    for i in range(N_CORES):
        w_shard = W16[:, i * N_SHARD : (i + 1) * N_SHARD]  # [D_IN, 512]
        # partition-major W: w_pm[p, k*512 + n] = W[k*128 + p, n0 + n]
        w_pm = np.ascontiguousarray(
            w_shard.reshape(K_TILES, 128, N_SHARD)
            .transpose(1, 0, 2)
            .reshape(128, K_TILES * N_SHARD)
        )
        b_shard = np.ascontiguousarray(
            bias[i * N_SHARD : (i + 1) * N_SHARD].reshape(N_TILES, 128).T
        )
        in_maps.append({"xT": xT_pm, "w": w_pm, "bias": b_shard})

    nc = _get_nc()
    res = run_bass_kernel_spmd(nc, in_maps, list(range(N_CORES)))
    out = np.empty((BATCH, UNITS), np.float32)
    for i in range(N_CORES):
        out[:, i * N_SHARD : (i + 1) * N_SHARD] = (
            res.results[i]["out"].astype(np.float32).T
        )
    return out
